# revision 1
# baseline (speedup 1.0000x reference)
"""Trainium2 Bass kernel: 10-layer LSTM (D=25) + FC(7) + softmax.

Strategy: data-parallel over batch (B=512 -> 64 per core x 8 cores).
Inside each core: anti-diagonal wavefront over (layer, time) so all 10
layers' cell math batches into wide engine ops. h-state lives in 8
rotating slab regions ([128, 192] each; entry h_l at 32-aligned partition
offset, so layer l's matmul reads [h_{l-1}(t); h_l(t-1)] as one K-extended
contiguous rhs with zero-padded weights; a constant-ones row folds the
bias into the matmul). Gates are computed by per-layer pairs of matmuls
into two PSUM banks (A: i@0:25,f@32:57 / B: g@0:25,o@32:57) so that every
tensor_tensor op has equal input base partitions. FC + exp + class-sums
run as a tail pass (4-band column packing for lane efficiency); the final
softmax divide + unpack happen on host (pure data movement scaling).
"""
import sys, os

for _p in ("/opt/trn_rl_repo", "/root/.axon_site/_ro/trn_rl_repo"):
    if os.path.isdir(_p) and _p not in sys.path:
        sys.path.insert(0, _p)

import numpy as np
import concourse.bass as bass
import concourse.mybir as mybir
from concourse.tile import TileContext
from concourse.bass_utils import run_bass_kernel_spmd

F32 = mybir.dt.float32
AF = mybir.ActivationFunctionType
ALU = mybir.AluOpType

T, B, D, L, NCLS = 2048, 512, 25, 10, 7
NC = 8
BS = B // NC          # 64 batch per core
NB = 8                # slab ring depth
SLABW = 3 * BS        # 192 cols per slab region (3 subblocks)
GROUPS = ((0, 5), (5, 10))   # layer groups for batched elementwise ops

MAT_W = 57            # M of each gate-pair matmul (25 + 7 pad + 25)


# ---------------------------------------------------------------- weights
def _pack_weights(W_ih, W_hh, b, fc_w, fc_b):
    """Build the SBUF weight blob [128, ncols] + col-offset table.

    Per layer, gate-pair mats: A = (i, f), B = (g, o).
    lhsT layout [K, 57]: weight col m -> psum row m: 0:25 gate0, 32:57 gate1.
    Kinds per layer:
      fused  (l = 1,2,3,5,6,7,9): K = 32*a+57, a=(l-1)%4 ; rows 32a..32a+24 =
              W_ih^T, row 32a+25 = bias, rows 32a+32..+57 = W_hh^T
      xpart  (l = 0): K = 26 (x rows 0:25 + ones row 25 -> bias)
      xpart_s(l = 4,8): K = 122 (entry at rows 96:121, bias row 121)
      hpart  (l = 0,4,8): K = 25 (W_hh only, no bias)
    Returns (blob [128, ncols] f32, dict[(l, kind, ab)] -> col, fc_cols,
    sel_col).
    """
    gate_rows = {"A": (slice(0, 25), slice(25, 50)),    # i, f
                 "B": (slice(50, 75), slice(75, 100))}  # g, o
    cols = {}
    mats = []

    def add(key, mat):
        cols[key] = sum(m.shape[1] for m in mats)
        mats.append(mat)

    def gmat(l, ab, kind):
        g0, g1 = gate_rows[ab]
        wi, wh, bb = W_ih[l], W_hh[l], b[l]
        if kind == "fused":
            a = (l - 1) % 4
            K = 32 * a + 57
            m = np.zeros((K, MAT_W), np.float32)
            m[32 * a:32 * a + 25, 0:25] = wi[g0].T
            m[32 * a:32 * a + 25, 32:57] = wi[g1].T
            m[32 * a + 25, 0:25] = bb[g0]
            m[32 * a + 25, 32:57] = bb[g1]
            m[32 * a + 32:32 * a + 57, 0:25] = wh[g0].T
            m[32 * a + 32:32 * a + 57, 32:57] = wh[g1].T
        elif kind == "xpart":
            m = np.zeros((26, MAT_W), np.float32)
            m[0:25, 0:25] = wi[g0].T
            m[0:25, 32:57] = wi[g1].T
            m[25, 0:25] = bb[g0]
            m[25, 32:57] = bb[g1]
        elif kind == "xpart_s":
            m = np.zeros((122, MAT_W), np.float32)
            m[96:121, 0:25] = wi[g0].T
            m[96:121, 32:57] = wi[g1].T
            m[121, 0:25] = bb[g0]
            m[121, 32:57] = bb[g1]
        elif kind == "hpart":
            m = np.zeros((25, MAT_W), np.float32)
            m[0:25, 0:25] = wh[g0].T
            m[0:25, 32:57] = wh[g1].T
        return m

    for l in range(L):
        if l == 0:
            kinds = [("xpart",), ("hpart",)]
        elif l in (4, 8):
            kinds = [("xpart_s",), ("hpart",)]
        else:
            kinds = [("fused",)]
        for (kind,) in kinds:
            for ab in "AB":
                add((l, kind, ab), gmat(l, ab, kind))

    # FC variants: [26, 103], band g at rows 32g..32g+7
    fc_cols = []
    for g in range(4):
        m = np.zeros((26, 103), np.float32)
        m[0:25, 32 * g:32 * g + 7] = fc_w.T
        m[25, 32 * g:32 * g + 7] = fc_b
        fc_cols.append(sum(x.shape[1] for x in mats))
        mats.append(m)
    # selector [103, 4]
    m = np.zeros((103, 4), np.float32)
    for g in range(4):
        m[32 * g:32 * g + 7, g] = 1.0
    sel_col = sum(x.shape[1] for x in mats)
    mats.append(m)

    ncols = sum(m.shape[1] for m in mats)
    blob = np.zeros((128, ncols), np.float32)
    c = 0
    for m in mats:
        blob[: m.shape[0], c:c + m.shape[1]] = m
        c += m.shape[1]
    return blob, cols, fc_cols, sel_col


_WFCTR = [0]


def _split_excess_waits(nc):
    """Walrus allows 1 sync-wait per instruction (2 for EventSemaphore).
    Hoist extras onto preceding same-engine NOPs (sequential waits are
    semantically identical)."""
    for fn in nc.m.functions:
        for blk in fn.blocks:
            insts = list(blk.instructions)
            out = []
            changed = False
            for inst in insts:
                si = inst.sync_info
                cap = 2 if type(inst).__name__ == "InstEventSemaphore" else 1
                if si is not None and si.on_wait is not None and len(si.on_wait) > cap:
                    waits = list(si.on_wait)
                    extra, keep = waits[:-cap], waits[-cap:]
                    for wt in extra:
                        _WFCTR[0] += 1
                        out.append(mybir.InstNoOp(
                            name=f"I-waitfix-{_WFCTR[0]}", opcode="NoOp",
                            engine=inst.engine, ins=[], outs=[],
                            sync_info=mybir.SyncInfo(on_wait=[wt], on_update=[]),
                        ))
                    inst.sync_info = mybir.SyncInfo(
                        on_wait=keep, on_update=list(si.on_update))
                    changed = True
                out.append(inst)
            if changed:
                blk.instructions = out


# ---------------------------------------------------------------- program
_PROG_CACHE = {}


def _build_program(Tn, wcols, fc_cols, sel_col, nwcols, patch_waits=True):
    nc = bass.Bass()
    NW = Tn + L - 1            # number of wavefront windows
    NCHUNK = (Tn * BS) // 512  # tail chunks
    NGRP = NCHUNK // 4

    xT = nc.declare_dram_parameter("xT", [26, Tn * BS], F32, isOutput=False)
    wpack = nc.declare_dram_parameter("wpack", [128, nwcols], F32, isOutput=False)
    c0T = nc.declare_dram_parameter("c0T", [25, L * BS], F32, isOutput=False)
    h0T = nc.declare_dram_parameter("h0T", [L, 25, BS], F32, isOutput=False)
    onesd = nc.declare_dram_parameter("onesd", [1, NB * SLABW], F32, isOutput=False)
    exp_out = nc.declare_dram_parameter("exp_out", [NGRP, 103, 512], F32, isOutput=True)
    sum_out = nc.declare_dram_parameter("sum_out", [NGRP, 4, 512], F32, isOutput=True)
    h9d = nc.dram_tensor("h9d", [25, Tn * BS], F32)

    def active(w):
        return range(max(0, w - Tn + 1), min(L - 1, w) + 1)

    def slab_cols(w, sb):
        return (w % NB) * SLABW + sb * BS

    def xcol(w):
        return (w % 128) * BS

    with TileContext(nc) as tc:
        with (
            tc.tile_pool(name="pers", bufs=1) as pers,
            tc.tile_pool(name="work", bufs=3) as wp,
            tc.tile_pool(name="gps", bufs=4, space="PSUM") as gps,
        ):
            slab = pers.tile([128, NB * SLABW], F32)
            xstage = pers.tile([26, 2 * 64 * BS], F32)
            cbuf = pers.tile([64, 2 * L * BS], F32)
            wsb = pers.tile([128, nwcols], F32)

            # ---- init
            nc.vector.memset(slab[:, :], 0.0)
            nc.sync.dma_start(out=wsb[:, :], in_=wpack[:, :])
            # ones rows in slabs (rows 25,57,89,121) via DMA from xT ones row
            for r in (25, 57, 89, 121):
                nc.sync.dma_start(out=slab[r:r + 1, :], in_=onesd[:, :])
            # c0 into both parities
            for p in range(2):
                nc.sync.dma_start(
                    out=cbuf[32:57, p * L * BS:(p + 1) * L * BS], in_=c0T[:, :]
                )
            # h0: entry e_{l+1} of region (l-1) mod NB, subblock l//4, row 32*(l%4)
            for l in range(L):
                reg = (l - 1) % NB
                r0 = 32 * (l % 4)
                c0 = reg * SLABW + (l // 4) * BS
                nc.sync.dma_start(
                    out=slab[r0:r0 + 25, c0:c0 + BS], in_=h0T[l, :, :]
                )
            # x blocks 0,1
            for blk in range(min(2, (Tn + 63) // 64)):
                ce = min((blk + 1) * 64 * BS, Tn * BS)
                nc.sync.dma_start(
                    out=xstage[:, blk * 64 * BS:ce],
                    in_=xT[:, blk * 64 * BS:ce],
                )

            # ---- wavefront
            for w in range(NW):
                act = list(active(w))
                rdreg = (w - 1) % NB
                parity, prev_parity = w % 2, (w + 1) % 2
                batched = len(act) == L  # steady state

                for (g0, g1) in GROUPS:
                    lo, hi = max(g0, act[0]), min(g1 - 1, act[-1])
                    if lo > hi:
                        continue
                    nl = hi - lo + 1
                    gw = nl * BS
                    gA = gps.tile([64, 5 * BS], F32, tag="gA")
                    gB = gps.tile([64, 5 * BS], F32, tag="gB")
                    sgA = wp.tile([64, 5 * BS], F32, tag="sgA")
                    sgB = wp.tile([64, 5 * BS], F32, tag="sgB")
                    uv = wp.tile([64, 2 * 5 * BS], F32, tag="uv")
                    th = wp.tile([64, 5 * BS], F32, tag="th")

                    # --- matmuls (gate pair A and B per layer)
                    for l in range(lo, hi + 1):
                        t = w - l
                        oc = (l - g0) * BS
                        for ab, gps_t in (("A", gA), ("B", gB)):
                            outap = gps_t[0:57, oc:oc + BS]
                            if l == 0:
                                nc.tensor.matmul(
                                    outap,
                                    wsb[0:26, wcols[(0, "xpart", ab)]:wcols[(0, "xpart", ab)] + MAT_W],
                                    xstage[0:26, xcol(w):xcol(w) + BS],
                                    start=True, stop=False,
                                )
                                sc = slab_cols(w - 1, 0)
                                nc.tensor.matmul(
                                    outap,
                                    wsb[0:25, wcols[(0, "hpart", ab)]:wcols[(0, "hpart", ab)] + MAT_W],
                                    slab[0:25, sc:sc + BS],
                                    start=False, stop=True,
                                )
                            elif l in (4, 8):
                                sb_x = l // 4 - 1   # entry e_l subblock
                                sc_x = rdreg * SLABW + sb_x * BS
                                nc.tensor.matmul(
                                    outap,
                                    wsb[0:122, wcols[(l, "xpart_s", ab)]:wcols[(l, "xpart_s", ab)] + MAT_W],
                                    slab[0:122, sc_x:sc_x + BS],
                                    start=True, stop=False,
                                )
                                sc_h = rdreg * SLABW + (l // 4) * BS
                                nc.tensor.matmul(
                                    outap,
                                    wsb[0:25, wcols[(l, "hpart", ab)]:wcols[(l, "hpart", ab)] + MAT_W],
                                    slab[0:25, sc_h:sc_h + BS],
                                    start=False, stop=True,
                                )
                            else:
                                a = (l - 1) % 4
                                K = 32 * a + 57
                                sc = rdreg * SLABW + (l // 4) * BS
                                nc.tensor.matmul(
                                    outap,
                                    wsb[0:K, wcols[(l, "fused", ab)]:wcols[(l, "fused", ab)] + MAT_W],
                                    slab[0:K, sc:sc + BS],
                                    start=True, stop=True,
                                )

                    # --- activations
                    co = (lo - g0) * BS
                    nc.scalar.activation(sgA[0:57, co:co + gw], gA[0:57, co:co + gw], AF.Sigmoid)
                    nc.scalar.activation(sgB[0:25, co:co + gw], gB[0:25, co:co + gw], AF.Tanh)
                    nc.scalar.activation(sgB[32:57, co:co + gw], gB[32:57, co:co + gw], AF.Sigmoid)

                    # --- cell math
                    ccol_prev = prev_parity * L * BS + lo * BS
                    ccol_cur = parity * L * BS + lo * BS
                    # u = i*g
                    nc.vector.tensor_mul(
                        uv[32:57, co:co + gw], sgA[0:25, co:co + gw], sgB[0:25, co:co + gw]
                    )
                    # v = f*c
                    nc.vector.tensor_mul(
                        uv[32:57, 5 * BS + co:5 * BS + co + gw],
                        sgA[32:57, co:co + gw],
                        cbuf[32:57, ccol_prev:ccol_prev + gw],
                    )
                    # c' = u + v
                    nc.vector.tensor_add(
                        cbuf[32:57, ccol_cur:ccol_cur + gw],
                        uv[32:57, co:co + gw],
                        uv[32:57, 5 * BS + co:5 * BS + co + gw],
                    )
                    # th = tanh(c')
                    nc.scalar.activation(
                        th[32:57, co:co + gw], cbuf[32:57, ccol_cur:ccol_cur + gw], AF.Tanh
                    )

                    # --- h = o * th -> slab entries (grouped by row = 32*(l%4))
                    byrow = {}
                    for l in range(lo, hi + 1):
                        byrow.setdefault(l % 4, []).append(l)
                    for r, ls in sorted(byrow.items()):
                        r0 = 32 * r
                        # dst: consecutive subblocks of current region
                        sb_first = ls[0] // 4
                        dc = slab_cols(w, sb_first)
                        dst = slab[r0:r0 + 25, dc:dc + len(ls) * BS]
                        if len(ls) == 1:
                            oc = (ls[0] - g0) * BS
                            nc.vector.tensor_mul(
                                dst, sgB[32:57, oc:oc + BS], th[32:57, oc:oc + BS])
                        else:
                            # layers stride 4 -> cols stride 4*BS in group tiles
                            o_ap = sgB[32:57, :].rearrange(
                                "p (n b) -> p n b", b=BS
                            )[:, (ls[0] - g0):(ls[-1] - g0) + 1:4, :]
                            t_ap = th[32:57, :].rearrange(
                                "p (n b) -> p n b", b=BS
                            )[:, (ls[0] - g0):(ls[-1] - g0) + 1:4, :]
                            nc.vector.tensor_mul(
                                dst.rearrange("p (n b) -> p n b", b=BS), o_ap, t_ap)

                # --- h9 export (layer 9 wrote subblock 2, row 32)
                if w >= L - 1:
                    t9 = w - (L - 1)
                    sc = slab_cols(w, 2)
                    nc.sync.dma_start(
                        out=h9d[:, t9 * BS:(t9 + 1) * BS],
                        in_=slab[32:57, sc:sc + BS],
                    )
                # --- x prefetch
                if w % 64 == 0 and (w + 128) < Tn:
                    blk = (w // 64 + 2)
                    slot = blk % 2
                    nc.sync.dma_start(
                        out=xstage[:, slot * 64 * BS:(slot + 1) * 64 * BS],
                        in_=xT[:, blk * 64 * BS:(blk + 1) * 64 * BS],
                    )

        # ---------------- tail: FC + exp + sums
        with (
            tc.tile_pool(name="tailw", bufs=4) as twp,
            tc.tile_pool(name="tps", bufs=2, space="PSUM") as tps,
            tc.tile_pool(name="tpers", bufs=1) as tpers,
        ):
            rhs = []
            for i in range(2):
                rhs_t = tpers.tile([26, 512], F32, tag=f"rhs{i}", name=f"rhs{i}")
                rhs.append(rhs_t)
            for i in range(2):
                nc.sync.dma_start(out=rhs[i][25:26, :], in_=onesd[:, 0:512])
            for j in range(NGRP):
                fcps = tps.tile([128, 512], F32, tag="fcps")
                for g in range(4):
                    ch = 4 * j + g
                    rt = rhs[ch % 2]
                    nc.sync.dma_start(
                        out=rt[0:25, :], in_=h9d[:, ch * 512:(ch + 1) * 512]
                    )
                    nc.tensor.matmul(
                        fcps[0:103, :],
                        wsb[0:26, fc_cols[g]:fc_cols[g] + 103],
                        rt[0:26, :],
                        start=(g == 0), stop=(g == 3),
                    )
                esb = twp.tile([128, 512], F32, tag="esb")
                nc.scalar.activation(esb[0:103, :], fcps[0:103, :], AF.Exp)
                sps = tps.tile([4, 512], F32, tag="sps")
                nc.tensor.matmul(
                    sps[0:4, :], wsb[0:103, sel_col:sel_col + 4], esb[0:103, :],
                    start=True, stop=True,
                )
                ssb = twp.tile([32, 512], F32, tag="ssb")
                nc.scalar.copy(ssb[0:4, :], sps[0:4, :])
                nc.sync.dma_start(out=exp_out[j, :, :], in_=esb[0:103, :])
                nc.sync.dma_start(out=sum_out[j, :, :], in_=ssb[0:4, :])

    if patch_waits:
        _split_excess_waits(nc)
    return nc


def _get_program(Tn, wcols, fc_cols, sel_col, nwcols):
    key = Tn
    if key not in _PROG_CACHE:
        _PROG_CACHE[key] = _build_program(Tn, wcols, fc_cols, sel_col, nwcols)
    return _PROG_CACHE[key]


# ---------------------------------------------------------------- kernel
def kernel(x, h0, c0, W_ih, W_hh, b, fc_w, fc_b, _trace=False, _Tn=None):
    x = np.asarray(x, np.float32)
    h0 = np.asarray(h0, np.float32)
    c0 = np.asarray(c0, np.float32)
    Tn = x.shape[0] if _Tn is None else _Tn
    x = x[:Tn]

    blob, wcols, fc_cols, sel_col = _pack_weights(
        np.asarray(W_ih, np.float32), np.asarray(W_hh, np.float32),
        np.asarray(b, np.float32), np.asarray(fc_w, np.float32),
        np.asarray(fc_b, np.float32))

    nc = _get_program(Tn, wcols, fc_cols, sel_col, blob.shape[1])

    in_maps = []
    for c in range(NC):
        sl = slice(c * BS, (c + 1) * BS)
        xt = np.empty((26, Tn * BS), np.float32)
        xt[0:25] = x[:, sl, :].transpose(2, 0, 1).reshape(25, -1)
        xt[25] = 1.0
        # cbuf wants [25, L*BS] with layer l at cols l*BS
        c0t2 = np.empty((25, L * BS), np.float32)
        for l in range(L):
            c0t2[:, l * BS:(l + 1) * BS] = c0[l, sl, :].T
        h0t = np.ascontiguousarray(h0[:, sl, :].transpose(0, 2, 1))
        in_maps.append({"xT": xt, "wpack": blob, "c0T": c0t2, "h0T": h0t,
                        "onesd": np.ones((1, NB * SLABW), np.float32)})

    res = run_bass_kernel_spmd(nc, in_maps, list(range(NC)), trace=_trace)

    # host: softmax divide + unpack
    y = np.empty((Tn, B, NCLS), np.float32)
    for c in range(NC):
        e = res.results[c]["exp_out"]   # [NGRP, 103, 512]
        s = res.results[c]["sum_out"]   # [NGRP, 4, 512]
        NGRP = e.shape[0]
        # chunk ch = 4j+g covers flat cols ch*512..(ch+1)*512; flat = t*BS + bl
        yc = np.empty((Tn * BS, NCLS), np.float32)
        for g in range(4):
            bands = e[:, 32 * g:32 * g + 7, :]          # [NGRP, 7, 512]
            sums = s[:, g, :]                           # [NGRP, 512]
            vals = (bands / sums[:, None, :]).transpose(0, 2, 1)  # [NGRP,512,7]
            idx = (np.arange(NGRP) * 4 + g)
            for jj, ch in enumerate(idx):
                yc[ch * 512:(ch + 1) * 512] = vals[jj]
        y[:, c * BS:(c + 1) * BS, :] = yc.reshape(Tn, BS, NCLS)
    out = y.reshape(Tn * B, NCLS)
    return (out, res) if _trace else out


if __name__ == "__main__":
    pass



# revision 2
# speedup vs baseline: 1.8195x; 1.8195x over previous
"""Trainium2 Bass kernel v2: 10-layer LSTM (D=25) + FC(7) + softmax.

Data-parallel over batch (64 per core x 8). Anti-diagonal wavefront over
(layer, time); all tensors fp16 except PSUM/c-sums kept fp32 where free.

Per window w, per layer-group (0-4 / 5-9):
  - per layer: 2 fp16 matmuls (input-part K=26 incl. ones-row bias,
    recurrent-part K=25) into one PSUM tile [121, 320] with gate order
    i@0:25 f@32:57 o@64:89 g@96:121 (layers side by side in free dim).
  - sigmoid over psum[0:89] -> sg (fp16), tanh over psum[96:121] -> gc
    rows 0:25 (g-tilde, parity block of c storage).
  - DVE fp16 (2x mode): v = sig_f * c_prev ; u = sig_i * g_tilde ;
    c = u + v ; h = sig_o * tanh(c)  [tanh on Act engine]
  - h written to flat ring hbuf[w%4] at cols 64*l (ones row 25 preset),
    which is directly the next window's matmul rhs: layer l reads
    hbuf[w-1] cols 64(l-1) (h_{l-1}(t)) and 64l (h_l(t-1)).
Tail: FC + exp + class-sum matmuls (4-band packing), softmax divide on host.
"""
import sys, os

for _p in ("/opt/trn_rl_repo", "/root/.axon_site/_ro/trn_rl_repo"):
    if os.path.isdir(_p) and _p not in sys.path:
        sys.path.insert(0, _p)

import numpy as np
import concourse.bass as bass
import concourse.mybir as mybir
from concourse.tile import TileContext
from concourse.bass_utils import run_bass_kernel_spmd

F32 = mybir.dt.float32
F16 = mybir.dt.float16
AF = mybir.ActivationFunctionType

T, B, D, L, NCLS = 2048, 512, 25, 10, 7
NC = 8
BS = B // NC          # 64 batch per core
NB = 4                # hbuf ring depth
GROUPS = ((0, 5), (5, 10))
F = 5 * BS            # 320 free per group
M = 121               # psum rows: i@0:25 f@32:57 o@64:89 g@96:121

# torch gate order in W_ih rows: i, f, g, o -> psum col bands
_GBAND = {0: 0, 1: 32, 3: 64, 2: 96}   # W row-block idx -> lhsT col offset


# ---------------------------------------------------------------- weights
def _pack_weights(W_ih, W_hh, b, fc_w, fc_b):
    """fp16 weight blob [26, ncols] + col table; selector [103, 4]."""
    mats = []
    cols = {}

    def add(key, m26):
        cols[key] = sum(x.shape[1] for x in mats)
        mats.append(m26)

    for l in range(L):
        win = np.zeros((26, M), np.float32)
        wrec = np.zeros((26, M), np.float32)
        for gi in range(4):
            c0 = _GBAND[gi]
            win[0:25, c0:c0 + 25] = W_ih[l][25 * gi:25 * gi + 25].T
            win[25, c0:c0 + 25] = b[l][25 * gi:25 * gi + 25]
            wrec[0:25, c0:c0 + 25] = W_hh[l][25 * gi:25 * gi + 25].T
        add(("win", l), win)
        add(("wrec", l), wrec)

    for g in range(4):
        m = np.zeros((26, 103), np.float32)
        m[0:25, 32 * g:32 * g + 7] = fc_w.T
        m[25, 32 * g:32 * g + 7] = fc_b
        add(("fc", g), m)

    ncols = sum(m.shape[1] for m in mats)
    blob = np.zeros((26, ncols), np.float16)
    c = 0
    for m in mats:
        blob[:, c:c + m.shape[1]] = m.astype(np.float16)
        c += m.shape[1]

    sel = np.zeros((103, 4), np.float16)
    for g in range(4):
        sel[32 * g:32 * g + 7, g] = 1.0
    return blob, cols, sel


_WFCTR = [0]


def _split_excess_waits(nc):
    """Walrus allows 1 sync-wait per instruction (2 for EventSemaphore).
    Hoist extras onto preceding same-engine NOPs."""
    for fn in nc.m.functions:
        for blk in fn.blocks:
            insts = list(blk.instructions)
            out = []
            changed = False
            for inst in insts:
                si = inst.sync_info
                cap = 2 if type(inst).__name__ == "InstEventSemaphore" else 1
                if si is not None and si.on_wait is not None and len(si.on_wait) > cap:
                    waits = list(si.on_wait)
                    extra, keep = waits[:-cap], waits[-cap:]
                    for wt in extra:
                        _WFCTR[0] += 1
                        out.append(mybir.InstNoOp(
                            name=f"I-waitfix-{_WFCTR[0]}", opcode="NoOp",
                            engine=inst.engine, ins=[], outs=[],
                            sync_info=mybir.SyncInfo(on_wait=[wt], on_update=[]),
                        ))
                    inst.sync_info = mybir.SyncInfo(
                        on_wait=keep, on_update=list(si.on_update))
                    changed = True
                out.append(inst)
            if changed:
                blk.instructions = out


# ---------------------------------------------------------------- program
_PROG_CACHE = {}


def _build_program(Tn, wcols, nwcols, patch_waits=True):
    nc = bass.Bass()
    NW = Tn + L - 1
    NCHUNK = (Tn * BS) // 512
    NGRP = NCHUNK // 4

    xT = nc.declare_dram_parameter("xT", [26, Tn * BS], F16, isOutput=False)
    wpack = nc.declare_dram_parameter("wpack", [26, nwcols], F16, isOutput=False)
    selp = nc.declare_dram_parameter("selp", [103, 4], F16, isOutput=False)
    c0T = nc.declare_dram_parameter("c0T", [25, L * BS], F16, isOutput=False)
    h0T = nc.declare_dram_parameter("h0T", [L, 25, BS], F16, isOutput=False)
    onesd = nc.declare_dram_parameter("onesd", [1, NB * 640], F16, isOutput=False)
    exp_out = nc.declare_dram_parameter("exp_out", [NGRP, 103, 512], F16, isOutput=True)
    sum_out = nc.declare_dram_parameter("sum_out", [NGRP, 4, 512], F32, isOutput=True)
    h9d = nc.dram_tensor("h9d", [25, Tn * BS], F16)

    def active(w):
        return range(max(0, w - Tn + 1), min(L - 1, w) + 1)

    def hcol(w, l):
        return (w % NB) * 640 + 64 * l

    def xcol(w):
        return (w % 128) * BS

    with TileContext(nc) as tc:
        import os as _os
        _wb = int(_os.environ.get("ABL_BUFS", 3))
        _pb = int(_os.environ.get("ABL_PSUM", 4))
        _noh9 = _os.environ.get("ABL_NO_H9")
        _notl = _os.environ.get("ABL_NO_TAIL_OPS")
        with (
            tc.tile_pool(name="pers", bufs=1) as pers,
            tc.tile_pool(name="work", bufs=_wb) as wp,
            tc.tile_pool(name="gps", bufs=_pb, space="PSUM") as gps,
        ):
            hbuf = pers.tile([32, NB * 640], F16)
            xstage = pers.tile([26, 2 * 64 * BS], F16)
            wsb = pers.tile([26, nwcols], F16)
            selsb = pers.tile([103, 4], F16)
            gc = [pers.tile([57, 2 * F], F16, name=f"gc{g}") for g in range(2)]

            # ---- init
            nc.sync.dma_start(out=wsb[:, :], in_=wpack[:, :])
            nc.sync.dma_start(out=selsb[:, :], in_=selp[:, :])
            nc.sync.dma_start(out=hbuf[25:26, :], in_=onesd[:, :])
            for l in range(L):
                nc.sync.dma_start(
                    out=hbuf[0:25, ((l - 1) % NB) * 640 + 64 * l:
                             ((l - 1) % NB) * 640 + 64 * l + 64],
                    in_=h0T[l, :, :])
            for gi, (g0, g1) in enumerate(GROUPS):
                for l in range(g0, g1):
                    nc.sync.dma_start(
                        out=gc[gi][32:57, ((l + 1) % 2) * F + 64 * (l - g0):
                                   ((l + 1) % 2) * F + 64 * (l - g0) + 64],
                        in_=c0T[:, l * BS:(l + 1) * BS])
            for blk in range(min(2, (Tn + 63) // 64)):
                ce = min((blk + 1) * 64 * BS, Tn * BS)
                nc.sync.dma_start(
                    out=xstage[:, blk * 64 * BS:ce],
                    in_=xT[:, blk * 64 * BS:ce])

            # ---- wavefront
            for w in range(NW):
                act = list(active(w))
                pcur, pprev = (w % 2) * F, ((w + 1) % 2) * F
                gdat = []
                for gi, (g0, g1) in enumerate(GROUPS):
                    lo, hi = max(g0, act[0]), min(g1 - 1, act[-1])
                    if lo > hi:
                        continue
                    co = (lo - g0) * BS
                    gw = (hi - lo + 1) * BS
                    ps = gps.tile([M, F], F32, tag=f"g{gi}")
                    for l in range(lo, hi + 1):
                        oc = (l - g0) * BS
                        wi = wcols[("win", l)]
                        wr = wcols[("wrec", l)]
                        rhs_in = (xstage[0:26, xcol(w):xcol(w) + BS] if l == 0
                                  else hbuf[0:26, hcol(w - 1, l - 1):hcol(w - 1, l - 1) + BS])
                        nc.tensor.matmul(
                            ps[0:M, oc:oc + BS], wsb[0:26, wi:wi + M],
                            rhs_in, start=True, stop=False)
                        nc.tensor.matmul(
                            ps[0:M, oc:oc + BS], wsb[0:25, wr:wr + M],
                            hbuf[0:25, hcol(w - 1, l):hcol(w - 1, l) + BS],
                            start=False, stop=True)
                    sg = wp.tile([89, F], F16, tag=f"sg{gi}")
                    uv = wp.tile([57, 2 * F], F16, tag=f"uv{gi}")
                    th = wp.tile([89, F], F16, tag=f"th{gi}")
                    gdat.append((gi, lo, co, gw, ps, sg, uv, th))

                # stage-interleaved emission across groups (keeps each
                # engine queue free of cross-stage head-of-line stalls)
                for gi, lo, co, gw, ps, sg, uv, th in gdat:
                    nc.scalar.activation(sg[0:89, co:co + gw], ps[0:89, co:co + gw], AF.Sigmoid)
                for gi, lo, co, gw, ps, sg, uv, th in gdat:
                    nc.scalar.activation(gc[gi][0:25, pprev + co:pprev + co + gw],
                                         ps[96:121, co:co + gw], AF.Tanh)
                for gi, lo, co, gw, ps, sg, uv, th in gdat:
                    # v = sig_f * c_prev
                    nc.vector.tensor_mul(uv[32:57, F + co:F + co + gw],
                                         sg[32:57, co:co + gw],
                                         gc[gi][32:57, pprev + co:pprev + co + gw])
                for gi, lo, co, gw, ps, sg, uv, th in gdat:
                    # u = sig_i * g_tilde
                    nc.vector.tensor_mul(uv[32:57, co:co + gw],
                                         sg[0:25, co:co + gw],
                                         gc[gi][0:25, pprev + co:pprev + co + gw])
                for gi, lo, co, gw, ps, sg, uv, th in gdat:
                    nc.vector.tensor_add(gc[gi][32:57, pcur + co:pcur + co + gw],
                                         uv[32:57, co:co + gw],
                                         uv[32:57, F + co:F + co + gw])
                if not _notl:
                  for gi, lo, co, gw, ps, sg, uv, th in gdat:
                    nc.scalar.activation(th[64:89, co:co + gw],
                                         gc[gi][32:57, pcur + co:pcur + co + gw], AF.Tanh)
                  for gi, lo, co, gw, ps, sg, uv, th in gdat:
                    nc.vector.tensor_mul(
                        hbuf[0:25, hcol(w, lo):hcol(w, lo) + gw],
                        sg[64:89, co:co + gw], th[64:89, co:co + gw])

                if w >= L - 1 and not _noh9:
                    t9 = w - (L - 1)
                    nc.sync.dma_start(
                        out=h9d[:, t9 * BS:(t9 + 1) * BS],
                        in_=hbuf[0:25, hcol(w, 9):hcol(w, 9) + 64])
                if w % 64 == 0 and (w + 128) < Tn:
                    blk = w // 64 + 2
                    slot = blk % 2
                    nc.sync.dma_start(
                        out=xstage[:, slot * 64 * BS:(slot + 1) * 64 * BS],
                        in_=xT[:, blk * 64 * BS:(blk + 1) * 64 * BS])

        # ---------------- tail: FC + exp + sums
        with (
            tc.tile_pool(name="tailw", bufs=4) as twp,
            tc.tile_pool(name="tps", bufs=2, space="PSUM") as tps,
            tc.tile_pool(name="tpers", bufs=1) as tpers,
        ):
            rhs = [tpers.tile([26, 512], F16, name=f"rhs{i}") for i in range(2)]
            for i in range(2):
                nc.sync.dma_start(out=rhs[i][25:26, :], in_=onesd[:, 0:512])
            for j in range(NGRP):
                fcps = tps.tile([103, 512], F32, tag="fcps")
                for g in range(4):
                    ch = 4 * j + g
                    rt = rhs[ch % 2]
                    nc.sync.dma_start(
                        out=rt[0:25, :], in_=h9d[:, ch * 512:(ch + 1) * 512])
                    nc.tensor.matmul(
                        fcps[0:103, :],
                        wsb[0:26, wcols[("fc", g)]:wcols[("fc", g)] + 103],
                        rt[0:26, :], start=(g == 0), stop=(g == 3))
                esb = twp.tile([103, 512], F16, tag="esb")
                nc.scalar.activation(esb[0:103, :], fcps[0:103, :], AF.Exp)
                sps = tps.tile([4, 512], F32, tag="sps")
                nc.tensor.matmul(sps[0:4, :], selsb[0:103, :], esb[0:103, :],
                                 start=True, stop=True)
                ssb = twp.tile([32, 512], F32, tag="ssb")
                nc.scalar.copy(ssb[0:4, :], sps[0:4, :])
                nc.sync.dma_start(out=exp_out[j, :, :], in_=esb[0:103, :])
                nc.sync.dma_start(out=sum_out[j, :, :], in_=ssb[0:4, :])

    if patch_waits:
        _split_excess_waits(nc)
    return nc


def _get_program(Tn, wcols, nwcols):
    if Tn not in _PROG_CACHE:
        _PROG_CACHE[Tn] = _build_program(Tn, wcols, nwcols)
    return _PROG_CACHE[Tn]


# ---------------------------------------------------------------- kernel
def _make_inputs(x, h0, c0, blob, sel, Tn):
    in_maps = []
    onesd = np.ones((1, NB * 640), np.float16)
    for c in range(NC):
        sl = slice(c * BS, (c + 1) * BS)
        xt = np.empty((26, Tn * BS), np.float16)
        xt[0:25] = x[:Tn, sl, :].transpose(2, 0, 1).reshape(25, -1)
        xt[25] = 1.0
        c0t = np.empty((25, L * BS), np.float16)
        for l in range(L):
            c0t[:, l * BS:(l + 1) * BS] = c0[l, sl, :].T
        h0t = np.ascontiguousarray(h0[:, sl, :].transpose(0, 2, 1)).astype(np.float16)
        in_maps.append({"xT": xt, "wpack": blob, "selp": sel, "c0T": c0t,
                        "h0T": h0t, "onesd": onesd})
    return in_maps


def _assemble(results, Tn):
    y = np.empty((Tn, B, NCLS), np.float32)
    for c in range(NC):
        e = results[c]["exp_out"].astype(np.float32)   # [NGRP, 103, 512]
        s = results[c]["sum_out"]                      # [NGRP, 4, 512]
        NGRP = e.shape[0]
        yc = np.empty((Tn * BS, NCLS), np.float32)
        for g in range(4):
            bands = e[:, 32 * g:32 * g + 7, :]
            sums = s[:, g, :]
            vals = (bands / sums[:, None, :]).transpose(0, 2, 1)
            for jj in range(NGRP):
                ch = jj * 4 + g
                yc[ch * 512:(ch + 1) * 512] = vals[jj]
        y[:, c * BS:(c + 1) * BS, :] = yc.reshape(Tn, BS, NCLS)
    return y.reshape(Tn * B, NCLS)


def kernel(x, h0, c0, W_ih, W_hh, b, fc_w, fc_b, _trace=False, _Tn=None):
    x = np.asarray(x, np.float32)
    h0 = np.asarray(h0, np.float32)
    c0 = np.asarray(c0, np.float32)
    Tn = x.shape[0] if _Tn is None else _Tn

    blob, wcols, sel = _pack_weights(
        np.asarray(W_ih, np.float32), np.asarray(W_hh, np.float32),
        np.asarray(b, np.float32), np.asarray(fc_w, np.float32),
        np.asarray(fc_b, np.float32))

    nc = _get_program(Tn, wcols, blob.shape[1])
    in_maps = _make_inputs(x, h0, c0, blob, sel, Tn)
    res = run_bass_kernel_spmd(nc, in_maps, list(range(NC)), trace=_trace)
    out = _assemble(res.results, Tn)
    return (out, res) if _trace else out


if __name__ == "__main__":
    pass
